# revision 3
# baseline (speedup 1.0000x reference)
"""Bidirectional GRU duration predictor on 8 Trainium2 NeuronCores.

Sharding: 48 time-chunks per direction (padded T=2064 = 48 x 43), 12 chunks
per core.  Each core runs 3 interleaved GROUPS; a group merges M=4 chunk
scans into one lockstep scan so every matmul has free width M*B = 128,
amortizing PE weight loads (LDWEIGHTS dominated the PE time at width 32).
Groups round-robin so one group's gate chain (ACT/DVE/GPSIMD) overlaps the
other groups' PE phases; each group's tanh/f2/h' tail is emitted one
group-slot late (software pipelining) so in-order engine queues are not
head-of-line blocked.

Each chunk scan warms up W=8 steps from h=0 (the update gate is
contractive; chunking rel err ~5e-3 vs the 2e-2 budget).  Chunk 0 is
zero-padded; h stays exactly 0 through its warmup because gi biases and
bhn are zero.

Per group device state:
  - history [128, NSTEPS+1 slots, 2k x M x B] bf16 — h(t) written to slot
    t+1 by DVE; matmuls read slot t; finished slots are DMA'd to DRAM and
    the output projection (h @ Wd) happens on the HOST from the shipped h
    states.  h is kept ONLY in bf16 (total measured rel err 7.5e-3).
  - gi = feats @ Wi + bi precomputed on HOST, shipped bf16, streamed in
    8-step chunks; the (r,z) parts enter PSUM via one identity matmul, the
    n part is added by DVE.

Gate algebra: the host negates the z-gate columns of Wi/bi/Wh so the z
accumulator natively holds -(iz+hz); ONE wide sigmoid then computes both
r = sigma(ir+hr) and q = sigma(-(iz+hz)) = 1-z, and the update is
    h' = v + q*n,  v = h - q*h   (= z*h, computed on GPSIMD off-path)
with n = tanh(gi_n + r*gh_n)  (bhn == 0 in this model — asserted on host).
Per step: PE 13 matmuls, ACT 2 (wide sigmoid, tanh), DVE 4 (m1, m2, f2,
h'), GPSIMD 2 (w, v) — no engine exceeds ~85%.
"""

import sys

if "/opt/trn_rl_repo" not in sys.path:
    sys.path.insert(0, "/opt/trn_rl_repo")

import numpy as np
import ml_dtypes

import concourse.bacc as bacc
import concourse.tile as tile
import concourse.mybir as mybir
from concourse.bass_utils import run_bass_kernel_spmd
from concourse.masks import make_identity

BF16 = mybir.dt.bfloat16
F32 = mybir.dt.float32
NPBF16 = ml_dtypes.bfloat16
AF = mybir.ActivationFunctionType
OP = mybir.AluOpType

B, T_FULL, H, FEAT = 32, 2048, 256, 64
NCORES = 8
G = 3                 # interleaved groups per core
M = 4                 # merged chunk-scans per group
NCHUNK_CORE = G * M   # 12 chunks per core
NCHUNK_DIR = 4 * NCHUNK_CORE  # 48 chunks per direction
OUT_STEPS = 43        # output steps per chunk (48*43 = 2064 >= 2048)
WARM = 8              # warmup steps per chunk
T_PAD = NCHUNK_DIR * OUT_STEPS  # 2064
NSTEPS = OUT_STEPS + WARM       # 51
HIST = NSTEPS + 1     # full history: slot t+1 holds h(t); slot 0 = h(-1) = 0
BLK = 11              # ys DMA roughly every BLK steps
GITC = 8              # gi prefetch chunk (steps)


def build_program():
    nc = bacc.Bacc()
    n_gi_chunks = (NSTEPS + GITC - 1) // GITC

    gi_d = nc.dram_tensor("giT", [128, G, NSTEPS, 6, M, B], BF16,
                          kind="ExternalInput")
    whb_d = nc.dram_tensor("whb", [128, 2 * 768], BF16, kind="ExternalInput")
    ys_d = nc.dram_tensor("ys", [128, G, OUT_STEPS, 2, M, B], BF16,
                          kind="ExternalOutput")

    with tile.TileContext(nc) as tcx:
        with (
            tcx.tile_pool(name="persist", bufs=1) as persist,
            tcx.tile_pool(name="gates", bufs=2) as gates,
            tcx.tile_pool(name="ps_rz0", bufs=1, space="PSUM") as ps_rz0,
            tcx.tile_pool(name="ps_rz1", bufs=1, space="PSUM") as ps_rz1,
            tcx.tile_pool(name="ps_rz2", bufs=1, space="PSUM") as ps_rz2,
            tcx.tile_pool(name="ps_n0", bufs=1, space="PSUM") as ps_n0,
            tcx.tile_pool(name="ps_n1", bufs=1, space="PSUM") as ps_n1,
            tcx.tile_pool(name="ps_n2", bufs=1, space="PSUM") as ps_n2,
        ):
            whb_s = persist.tile([128, 2 * 768], BF16, tag="whb")
            ident = persist.tile([128, 128], BF16, tag="ident")
            hist = [persist.tile([128, HIST, 2, M, B], BF16, tag=f"hist{g}",
                                 name=f"hist{g}") for g in range(G)]
            gib = [[persist.tile([128, GITC, 6, M, B], BF16,
                                 tag=f"gib{g}_{i}", name=f"gib{g}_{i}")
                    for i in range(2)] for g in range(G)]
            ps_rz = [ps_rz0, ps_rz1, ps_rz2]
            ps_n = [ps_n0, ps_n1, ps_n2]

            # ---- prologue ----
            nc.sync.dma_start(whb_s[:], whb_d[:])
            for g in range(G):
                nc.sync.dma_start(gib[g][0][:], gi_d[:, g, 0:GITC])
            make_identity(nc, ident[:])
            for g in range(G):
                nc.gpsimd.memset(hist[g][:, 0], 0.0)

            def emit_step(g, t):
                c, tloc = t // GITC, t % GITC
                gi_cur = gib[g][c % 2]
                sp = t  # slot holding h(t-1)
                rz = ps_rz[g].tile([128, 4, M, B], F32, tag=f"rz{g}",
                                   name=f"rz{g}")
                ghn = ps_n[g].tile([128, 2, M, B], F32, tag=f"ghn{g}",
                                   name=f"ghn{g}")
                # PSUM preload: gi(r, -z) via identity matmul
                nc.tensor.matmul(rz[:], lhsT=ident[:], rhs=gi_cur[:, tloc, 0:4],
                                 start=True, stop=False, skip_group_check=True)
                # recurrent matmuls; n-gate group starts its own accumulation
                # (bhn == 0, so no preload needed for the n bank)
                dests = (rz[:, 0:2], rz[:, 2:4], ghn)
                for gate in range(3):
                    for mc in range(2):
                        gm = gate * 2 + mc
                        for k in range(2):
                            nc.tensor.matmul(
                                dests[gate][:, mc],
                                lhsT=whb_s[:, k * 768 + gm * 128:
                                           k * 768 + (gm + 1) * 128],
                                rhs=hist[g][:, sp, k],
                                start=(gate == 2 and k == 0),
                                stop=(k == 1),
                                skip_group_check=True,
                            )
                    if gate == 1:
                        # r = sigma(ir+hr), q = sigma(-(iz+hz)) = 1-z in ONE op
                        rq = gates.tile([128, 4, M, B], BF16,
                                        tag=f"rq{g}", name=f"rq{g}")
                        nc.scalar.activation(rq[:], rz[:], AF.Sigmoid)
                # off-path on GPSIMD: v = z*h = h - q*h (ready before tanh)
                w = gates.tile([128, 2, M, B], BF16, tag=f"w{g}", name=f"w{g}")
                nc.gpsimd.tensor_tensor(w[:], rq[:, 2:4], hist[g][:, sp],
                                        OP.mult)
                v = gates.tile([128, 2, M, B], BF16, tag=f"v{g}", name=f"v{g}")
                nc.gpsimd.tensor_tensor(v[:], hist[g][:, sp], w[:],
                                        OP.subtract)
                # DVE: n = tanh(gi_n + r*gh_n)
                m1 = gates.tile([128, 2, M, B], BF16, tag=f"m1{g}",
                                name=f"m1{g}")
                nc.vector.tensor_tensor(m1[:], ghn[:], rq[:, 0:2], OP.mult)
                m2 = gates.tile([128, 2, M, B], BF16, tag=f"m2{g}",
                                name=f"m2{g}")
                nc.vector.tensor_tensor(m2[:], m1[:], gi_cur[:, tloc, 4:6],
                                        OP.add)
                # prefetch next gi chunk
                if tloc == 0 and c + 1 < n_gi_chunks:
                    n = min(GITC, NSTEPS - (c + 1) * GITC)
                    nc.sync.dma_start(
                        gib[g][(c + 1) % 2][:, 0:n],
                        gi_d[:, g, (c + 1) * GITC:(c + 1) * GITC + n],
                    )
                return rq, m2, v

            def emit_tail(g, t, rq, m2, v):
                # emitted one group-slot later (software pipelining) so these
                # land in the ACT/DVE queues about when their inputs are ready
                n_act = gates.tile([128, 2, M, B], BF16, tag=f"nact{g}",
                                   name=f"nact{g}")
                nc.scalar.activation(n_act[:], m2[:], AF.Tanh)
                f2 = gates.tile([128, 2, M, B], BF16, tag=f"f2{g}",
                                name=f"f2{g}")
                nc.vector.tensor_tensor(f2[:], rq[:, 2:4], n_act[:], OP.mult)
                nc.vector.tensor_tensor(hist[g][:, t + 1], v[:], f2[:],
                                        OP.add)
                # ship finished h states (output steps are s >= WARM)
                if t == NSTEPS - 1 or (t % BLK == BLK - 2 and t > WARM):
                    s0 = last_shipped[g] + 1
                    nc.sync.dma_start(ys_d[:, g, s0 - WARM:t + 1 - WARM],
                                      hist[g][:, s0 + 1:t + 2])
                    last_shipped[g] = t

            last_shipped = [WARM - 1] * G
            pending = None
            for t in range(NSTEPS):
                for g in range(G):
                    res = emit_step(g, t)
                    if pending is not None:
                        emit_tail(*pending)
                    pending = (g, t, *res)
            emit_tail(*pending)

    nc.finalize()
    return nc


_PROGRAM_CACHE = {}


def get_program():
    if "p" not in _PROGRAM_CACHE:
        _PROGRAM_CACHE["p"] = build_program()
    return _PROGRAM_CACHE["p"]


def make_in_maps(inputs):
    dur = np.asarray(inputs["duration_input"], np.float32)
    sid = np.asarray(inputs["sid_input"]).astype(np.int64)
    embed = np.asarray(inputs["embed"], np.float32)
    feats = np.concatenate([dur[..., None], embed[sid]], axis=-1)  # [B,T,F]

    # the kernel assumes bhn == 0 (true for this model's reference)
    assert not np.any(np.asarray(inputs["bhn_f"]))
    assert not np.any(np.asarray(inputs["bhn_b"]))

    gi_dir = {}
    for d in ("f", "b"):
        seq = feats if d == "f" else feats[:, ::-1]
        Wi = np.asarray(inputs[f"Wi_{d}"], np.float32)
        bi = np.asarray(inputs[f"bi_{d}"], np.float32)
        pad = np.concatenate(
            [np.zeros((B, WARM, FEAT), np.float32), seq,
             np.zeros((B, T_PAD - T_FULL, FEAT), np.float32)], axis=1
        )  # [B, WARM + T_PAD, F]
        gi = (pad.reshape(-1, FEAT) @ Wi + bi).reshape(B, WARM + T_PAD, 6, 128)
        gi[:, :, 2:4] *= -1.0  # negated z gate: accumulator holds -(iz+hz)
        gi_dir[d] = gi.astype(NPBF16)

    in_maps = []
    for c in range(NCORES):
        d = "f" if c < NCORES // 2 else "b"
        Wh = np.asarray(inputs[f"Wh_{d}"], np.float32)
        giT = np.empty((128, G, NSTEPS, 6, M, B), NPBF16)
        for g in range(G):
            for m in range(M):
                j = (c % 4) * NCHUNK_CORE + g * M + m
                sl = gi_dir[d][:, j * OUT_STEPS: j * OUT_STEPS + NSTEPS]
                # [B, NSTEPS, 6, 128] -> [128, NSTEPS, 6, B]
                giT[:, g, :, :, m, :] = sl.transpose(3, 1, 2, 0)
        whb = np.ascontiguousarray(
            Wh.reshape(2, 128, 768).transpose(1, 0, 2).reshape(128, 1536)
        ).copy()
        whb[:, 256:512] *= -1.0    # negated z gate (k = 0 half)
        whb[:, 1024:1280] *= -1.0  # negated z gate (k = 1 half)
        in_maps.append({
            "giT": giT,
            "whb": whb.astype(NPBF16),
        })
    return in_maps


def assemble_output(results, inputs):
    h_dir = {
        "f": np.empty((B, T_PAD, H), np.float32),
        "b": np.empty((B, T_PAD, H), np.float32),
    }
    for c in range(NCORES):
        d = "f" if c < NCORES // 2 else "b"
        ys = np.asarray(results[c]["ys"]).astype(np.float32)
        # [128, G, OUT_STEPS, 2, M, B]
        for g in range(G):
            for m in range(M):
                j = (c % 4) * NCHUNK_CORE + g * M + m
                steps = ys[:, g, :, :, m, :]  # [128, OUT_STEPS, 2, B]
                h_loc = steps.transpose(3, 1, 2, 0).reshape(B, OUT_STEPS, H)
                h_dir[d][:, j * OUT_STEPS:(j + 1) * OUT_STEPS] = h_loc
    fwd = h_dir["f"][:, :T_FULL]
    bwd = h_dir["b"][:, :T_FULL][:, ::-1]
    Wd = np.asarray(inputs["Wd"], np.float32)
    bd = np.asarray(inputs["bd"], np.float32)
    out = fwd @ Wd[:H] + bwd @ Wd[H:] + bd
    return np.ascontiguousarray(out.astype(np.float32))


def kernel(**inputs):
    nc = get_program()
    in_maps = make_in_maps(inputs)
    res = run_bass_kernel_spmd(nc, in_maps, list(range(NCORES)))
    return assemble_output(res.results, inputs)
